# revision 36
# baseline (speedup 1.0000x reference)
"""Trainium2 Bass kernel for nn_ModelAttention2Layers (B=8, S=2048, D=512, K=256).

Key structural insight: the reference returns final[0, -1, :] — only batch 0
matters (attention is independent per batch element), so batches 1-7 are dead
compute. Strategy: shard the 2048-query sequence of batch 0 across the 8
cores (256 queries each), with:
  - block 1 fully local per core (xT replicated -> k1T computed redundantly,
    so block 1 needs zero collectives)
  - one AllGather of the {k2T, v2} shards for block 2
  - block 3 flash-style: tiny AllGather of hidden[-1], per-core partial
    softmax/AV over the local 256 keys, one tiny AllReduce of [o|l]
Matmuls run in float32r (full-rate PE, ~11-bit mantissa); softmax statistics,
normalization and reductions in float32. k-projection biases are dropped —
they shift each query's logits by a per-query constant, which softmax
cancels exactly.
"""
import sys

sys.path.insert(0, "/opt/trn_rl_repo")

import numpy as np

S, D, K, P, C = 2048, 512, 256, 128, 8
SH = S // C          # 256 queries/keys per core
ND, NK, NS, NSH = D // P, K // P, S // P, SH // P   # 4, 2, 16, 2

_cache = {}


def _build():
    import concourse.bass as bass
    import concourse.tile as tile
    from concourse import mybir, bacc

    F32 = mybir.dt.float32
    F32R = mybir.dt.float32r
    BF16 = mybir.dt.bfloat16
    AF = mybir.ActivationFunctionType
    ts = bass.ts

    nc = bacc.Bacc()

    ins = {}
    for name, shape in [
        ("xT", [D, S]), ("x0", [S, D]), ("xTq", [D, SH]),
        ("Wk1", [D, K]), ("Wq1", [D, K]), ("Wk2", [D, K]), ("Wq2", [D, K]),
        ("Wv2", [D, D]), ("bq1", [K]), ("bq2", [K]), ("bv2row", [1, D]),
        ("ones", [1, P]), ("onescol", [P, 1]), ("ident", [P, P]),
    ]:
        ins[name] = nc.dram_tensor(name, shape, F32, kind="ExternalInput")
    out_ext = nc.dram_tensor("out", [D], F32, kind="ExternalOutput")

    GA = NK * P * K + NK * P * D   # gather-A floats per core: k2T + v2 shards

    with tile.TileContext(nc) as tc:
        with tc.tile_pool(name="const", bufs=1) as cw, \
             tc.tile_pool(name="big", bufs=1) as big, \
             tc.tile_pool(name="work", bufs=1) as wk, \
             tc.tile_pool(name="pp", bufs=2) as pp, \
             tc.tile_pool(name="small", bufs=2) as sm, \
             tc.tile_pool(name="stage", bufs=2) as stg, \
             tc.tile_pool(name="ps", bufs=1, space="PSUM") as ps, \
             tc.tile_pool(name="dram", bufs=1, space="DRAM") as dram, \
             tc.tile_pool(name="shdram", bufs=1, space="DRAM") as shd:

            # ---- input loads ----
            # small weights cast-load via gpsimd; bulk tensors (xT, x0) load as
            # f32 on sync HWDGE queues and cast to f32r on DVE (parallel paths,
            # so the first k1T matmul can start within a few us)
            W_r = {}
            for w, ncol in [("Wk1", K), ("Wq1", K)]:
                W_r[w] = cw.tile([P, ND, ncol], F32R, name=f"W_{w}", tag=f"W_{w}")
                nc.gpsimd.dma_start(W_r[w][:], ins[w][:].rearrange("(k p) n -> p k n", p=P))
            xT_r = big.tile([P, ND, S], F32R, tag="XV")
            for k in range(ND):
                st = stg.tile([P, S], F32, tag="stg")
                nc.sync.dma_start(
                    st[:], ins["xT"][:].rearrange("(k2 p) s -> p k2 s", p=P)[:, k, :])
                nc.vector.tensor_copy(xT_r[:, k, :], st[:])
            xTq_r = cw.tile([P, ND, SH], F32R)
            nc.gpsimd.dma_start(xTq_r[:], ins["xTq"][:].rearrange("(k p) j -> p k j", p=P))
            x0_r = cw.tile([P, NS, D], F32R)
            for n4 in range(4):
                st = stg.tile([P, S], F32, tag="stg")
                nc.sync.dma_start(
                    st[:].rearrange("p (n d) -> p n d", n=4),
                    ins["x0"][:].rearrange("(n p) d -> p n d", p=P)[:, 4 * n4:4 * n4 + 4, :])
                nc.vector.tensor_copy(
                    x0_r[:, 4 * n4:4 * n4 + 4, :].rearrange("p n d -> p (n d)"), st[:])
            for w, ncol in [("Wk2", K), ("Wq2", K), ("Wv2", D)]:
                W_r[w] = cw.tile([P, ND, ncol], F32R, name=f"W_{w}", tag=f"W_{w}")
                st = stg.tile([P, ND * ncol], F32, tag="stg", name=f"st_{w}")
                nc.sync.dma_start(
                    st[:].rearrange("p (k n) -> p k n", k=ND),
                    ins[w][:].rearrange("(k p) n -> p k n", p=P))
                nc.vector.tensor_copy(
                    W_r[w][:].rearrange("p k n -> p (k n)"), st[:])
            bq1_sb = cw.tile([P, NK], F32)
            nc.sync.dma_start(bq1_sb[:], ins["bq1"][:].rearrange("(m p) -> p m", p=P))
            bq2_sb = cw.tile([P, NK], F32)
            nc.sync.dma_start(bq2_sb[:], ins["bq2"][:].rearrange("(m p) -> p m", p=P))
            bv2_r = cw.tile([1, D], F32R)
            nc.gpsimd.dma_start(bv2_r[:], ins["bv2row"][:])
            ones_r = cw.tile([1, P], F32R)
            nc.gpsimd.dma_start(ones_r[:], ins["ones"][:])
            ident_r = cw.tile([P, P], F32R)
            nc.gpsimd.dma_start(ident_r[:], ins["ident"][:])
            Wq2f = cw.tile([P, ND, K], F32)
            nc.sync.dma_start(Wq2f[:], ins["Wq2"][:].rearrange("(k p) n -> p k n", p=P))
            onescol_f = cw.tile([P, 1], F32)
            nc.sync.dma_start(onescol_f[:], ins["onescol"][:])

            # ---- block 1 projections ----
            # k1T full [K, S], computed redundantly on every core (no bias: softmax-invariant)
            k1T = big.tile([P, NK, S], F32R, tag="kT")
            for m in range(NK):
                for cb in range(S // 512):
                    pm = ps.tile([P, 512], F32, tag="mm")
                    for k in range(ND):
                        nc.tensor.matmul(pm[:], W_r["Wk1"][:, k, ts(m, P)],
                                         xT_r[:, k, ts(cb, 512)],
                                         start=(k == 0), stop=(k == ND - 1))
                    nc.vector.tensor_copy(k1T[:, m, ts(cb, 512)], pm[:])
            # q1T shard [K, SH] with bias bq1
            q1T = wk.tile([P, NK, SH], F32R, tag="qT")
            for m in range(NK):
                pm = ps.tile([P, SH], F32, tag="mm")
                for k in range(ND):
                    nc.tensor.matmul(pm[:], W_r["Wq1"][:, k, ts(m, P)], xTq_r[:, k, :],
                                     start=(k == 0), stop=(k == ND - 1))
                nc.vector.tensor_scalar_add(q1T[:, m, :], pm[:], bq1_sb[:, m:m + 1])

            def attention(qT, kT_full, V_full, out_dst, pt_dtype):
                """out_dst[:, qm, :] = softmax(q.k^T) @ V for this core's 256 queries."""
                for qm in range(NSH):
                    sc = ps.tile([P, 4, 512], F32, tag="sc")
                    for ks in range(4):
                        for dm in range(NK):
                            nc.tensor.matmul(sc[:, ks, :], qT[:, dm, ts(qm, P)],
                                             kT_full[:, dm, ts(ks, 512)],
                                             start=(dm == 0), stop=(dm == NK - 1))
                    mx = sm.tile([P, 1], F32, tag="mx")
                    nc.vector.reduce_max(mx[:], sc[:], axis=mybir.AxisListType.XY)
                    nm = sm.tile([P, 1], F32, tag="nm")
                    nc.vector.tensor_scalar_mul(nm[:], mx[:], -1.0)
                    Pt = pp.tile([P, S], F32R, tag="P")
                    lsum = sm.tile([P, 4], F32, tag="lsum")
                    for ks in range(4):
                        nc.scalar.activation(Pt[:, ts(ks, 512)], sc[:, ks, :], AF.Exp,
                                             bias=nm[:], accum_out=lsum[:, ks:ks + 1])
                    l = sm.tile([P, 1], F32, tag="l")
                    nc.vector.reduce_sum(l[:], lsum[:], axis=mybir.AxisListType.X)
                    rl = sm.tile([P, 1], F32, tag="rl")
                    nc.vector.reciprocal(rl[:], l[:])
                    PT = pp.tile([P, NS, P], pt_dtype, tag="PT")
                    for n in range(NS):
                        tp = ps.tile([P, P], F32R, tag="tp")
                        nc.tensor.transpose(tp[:], Pt[:, ts(n, P)], ident_r[:])
                        nc.vector.tensor_copy(PT[:, n, :], tp[:])
                    av = ps.tile([P, D], F32, tag="mm")
                    for n in range(NS):
                        nc.tensor.matmul(av[:], PT[:, n, :], V_full[:, n, :],
                                         start=(n == 0), stop=(n == NS - 1))
                    nc.scalar.activation(out_dst[:, qm, :], av[:], AF.Copy, scale=rl[:])

            out1 = wk.tile([P, NSH, D], F32R, tag="H")
            attention(q1T, k1T, x0_r, out1, F32R)

            def transpose_rows(src, ncols_chunks):
                """src [P, NSH, D] -> dst [P, ND, SH] (row-major shard transposed)."""
                dst = wk.tile([P, ND, SH], F32R, tag="HT")
                for qm in reversed(range(NSH)):
                    for dm in range(ND):
                        tp = ps.tile([P, P], F32R, tag="tp")
                        nc.tensor.transpose(tp[:], src[:, qm, ts(dm, P)], ident_r[:])
                        nc.vector.tensor_copy(dst[:, dm, ts(qm, P)], tp[:])
                return dst

            out1T = transpose_rows(out1, ND)

            # ---- block 2 shard projections ----
            k2T = wk.tile([P, NK, SH], BF16, tag="kv_k")
            for m in range(NK):
                pm = ps.tile([P, SH], F32, tag="mm")
                for k in range(ND):
                    nc.tensor.matmul(pm[:], W_r["Wk2"][:, k, ts(m, P)], out1T[:, k, :],
                                     start=(k == 0), stop=(k == ND - 1))
                nc.vector.tensor_copy(k2T[:, m, :], pm[:])
            # gather 1 (k2T) fires while q2T/v2 are still being computed
            gk_in = dram.tile([NK * P * SH], BF16)
            nc.sync.dma_start(
                gk_in[:].rearrange("(m p j) -> p m j", m=NK, p=P), k2T[:])
            gk_out = shd.tile([C, NK * P * SH], BF16, addr_space="Shared")
            nc.gpsimd.collective_compute(
                "AllGather", mybir.AluOpType.bypass,
                replica_groups=[list(range(C))],
                ins=[gk_in.opt()], outs=[gk_out.opt()],
            )
            k2T_full = big.tile([P, NK, S], BF16, tag="kT")
            for m in range(NK):
                nc.sync.dma_start(
                    k2T_full[:, m, :].rearrange("p (c j) -> p c j", c=C),
                    gk_out[:, m * P * SH:(m + 1) * P * SH].rearrange(
                        "c (p j) -> p c j", p=P))
            q2T = wk.tile([P, NK, SH], BF16, tag="qT")
            for m in range(NK):
                pm = ps.tile([P, SH], F32, tag="mm")
                for k in range(ND):
                    nc.tensor.matmul(pm[:], W_r["Wq2"][:, k, ts(m, P)], out1T[:, k, :],
                                     start=(k == 0), stop=(k == ND - 1))
                nc.vector.tensor_scalar_add(q2T[:, m, :], pm[:], bq2_sb[:, m:m + 1])

            def vproj_norm(hT, out_dtype):
                """v = normalize_rows(h @ Wv2 + bv2) for this core's 256 rows."""
                v_sb = wk.tile([P, NSH, D], out_dtype, tag="kv_v")
                for r in range(NSH):
                    pm = ps.tile([P, D], F32, tag="mm")
                    for k in range(ND):
                        nc.tensor.matmul(pm[:], hT[:, k, ts(r, P)], W_r["Wv2"][:, k, :],
                                         start=(k == 0), stop=False)
                    nc.tensor.matmul(pm[:], ones_r[:], bv2_r[:], start=False, stop=True)
                    scr = sm.tile([P, D], F32, tag="scr")
                    ssum = sm.tile([P, 1], F32, tag="ssum")
                    nc.scalar.activation(scr[:], pm[:], AF.Square, accum_out=ssum[:])
                    nrm = sm.tile([P, 1], F32, tag="nrm")
                    nc.scalar.sqrt(nrm[:], ssum[:])
                    rn = sm.tile([P, 1], F32, tag="rn")
                    nc.vector.reciprocal(rn[:], nrm[:])
                    nc.scalar.activation(v_sb[:, r, :], pm[:], AF.Copy, scale=rn[:])
                return v_sb

            v2 = vproj_norm(out1T, BF16)

            gv_in = dram.tile([NSH * P * D], BF16)
            nc.sync.dma_start(
                gv_in[:].rearrange("(r p d) -> p r d", r=NSH, p=P), v2[:])
            gv_out = shd.tile([C, NSH * P * D], BF16, addr_space="Shared")
            nc.gpsimd.collective_compute(
                "AllGather", mybir.AluOpType.bypass,
                replica_groups=[list(range(C))],
                ins=[gv_in.opt()], outs=[gv_out.opt()],
            )
            v2_full = big.tile([P, NS, D], BF16, tag="XV")
            for r in range(NSH):
                off = r * P * D
                nc.sync.dma_start(
                    v2_full[:].rearrange("p (c r) d -> p c r d", c=C)[:, :, r, :],
                    gv_out[:, off:off + P * D].rearrange("c (p d) -> p c d", p=P))

            # ---- block 2 attention ----
            hidden = wk.tile([P, NSH, D], F32R, tag="H")
            attention(q2T, k2T_full, v2_full, hidden, BF16)
            hT = transpose_rows(hidden, ND)

            # broadcast hidden[-1] (core 7's last local row)
            gB_in = dram.tile([D], F32)
            nc.gpsimd.dma_start(gB_in[:].rearrange("(dm p) -> p dm", p=P), hT[:, :, SH - 1])
            gB_out = shd.tile([C, D], F32, addr_space="Shared")
            nc.gpsimd.collective_compute(
                "AllGather", mybir.AluOpType.bypass,
                replica_groups=[list(range(C))],
                ins=[gB_in.opt()], outs=[gB_out.opt()],
            )
            hl_r = sm.tile([P, ND], F32, tag="hl")
            nc.sync.dma_start(hl_r[:], gB_out[C - 1, :].rearrange("(dm p) -> p dm", p=P))

            # ---- block 3 (flash-style partials over this core's 256 keys) ----
            k3T = wk.tile([P, NK, SH], F32, tag="kv_k")
            for m in range(NK):
                pm = ps.tile([P, SH], F32, tag="mm")
                for k in range(ND):
                    nc.tensor.matmul(pm[:], W_r["Wk2"][:, k, ts(m, P)], hT[:, k, :],
                                     start=(k == 0), stop=(k == ND - 1))
                nc.vector.tensor_copy(k3T[:, m, :], pm[:])
            v3 = vproj_norm(hT, F32)

            # q3 = Wq2^T @ h_last + bq2
            q3 = sm.tile([P, NK], F32, tag="q3")
            for fm in range(NK):
                pm = ps.tile([P, 1], F32, tag="mm")
                for dm in range(ND):
                    nc.tensor.matmul(pm[:], Wq2f[:, dm, ts(fm, P)], hl_r[:, dm:dm + 1],
                                     start=(dm == 0), stop=(dm == ND - 1))
                nc.vector.tensor_scalar_add(q3[:, fm:fm + 1], pm[:], bq2_sb[:, fm:fm + 1])

            # s3 (scores for my 256 keys; |s3| <= ~4 so exp needs no max shift)
            s3p = ps.tile([P, NSH], F32, tag="tp")
            for n in range(NSH):
                for fm in range(NK):
                    nc.tensor.matmul(s3p[:, n:n + 1], k3T[:, fm, ts(n, P)], q3[:, fm:fm + 1],
                                     start=(fm == 0), stop=(fm == NK - 1))
            p3 = sm.tile([P, NSH], F32, tag="p3")
            nc.scalar.activation(p3[:], s3p[:], AF.Exp)

            # partial numerator o3 = p3 @ v3 and partial denominator l3 = sum p3
            o3p = ps.tile([1, D], F32, tag="mm")
            for n in range(NSH):
                nc.tensor.matmul(o3p[:], p3[:, n:n + 1], v3[:, n, :],
                                 start=(n == 0), stop=(n == NSH - 1))
            l3p = ps.tile([1, 1], F32, tag="tp")
            for n in range(NSH):
                nc.tensor.matmul(l3p[:], p3[:, n:n + 1], onescol_f[:],
                                 start=(n == 0), stop=(n == NSH - 1))
            ol = wk.tile([1, D + 1], F32, tag="ol")
            nc.vector.tensor_copy(ol[:, 0:D], o3p[:])
            nc.vector.tensor_copy(ol[:, D:D + 1], l3p[:])

            ar_in = dram.tile([1, D + 1], F32)
            nc.sync.dma_start(ar_in[:], ol[:])
            ar_out = shd.tile([C, D + 1], F32, addr_space="Shared")
            nc.gpsimd.collective_compute(
                "AllGather", mybir.AluOpType.bypass,
                replica_groups=[list(range(C))],
                ins=[ar_in.opt()], outs=[ar_out.opt()],
            )
            rb = wk.tile([1, D + 1, C], F32, tag="rb")
            nc.sync.dma_start(rb[:], ar_out[:].rearrange("c (o e) -> o e c", o=1))
            tot = wk.tile([1, D + 1], F32, tag="tot")
            nc.vector.reduce_sum(tot[:], rb[:], axis=mybir.AxisListType.X)
            rl3 = sm.tile([1, 1], F32, tag="rl3")
            nc.vector.reciprocal(rl3[:], tot[:, D:D + 1])
            fin = wk.tile([1, D], F32, tag="fin")
            nc.vector.tensor_scalar_mul(fin[:], tot[:, 0:D], rl3[:])
            nc.sync.dma_start(out_ext[:].rearrange("(a b) -> a b", a=1), fin[:])

    nc.finalize()
    return nc


def kernel(**inputs):
    from concourse.bass_utils import run_bass_kernel_spmd

    f = lambda k: np.ascontiguousarray(np.asarray(inputs[k], dtype=np.float32))
    x0 = f("x")[0]                       # [S, D]; batches 1..7 are dead
    xT = np.ascontiguousarray(x0.T)      # [D, S]
    base = {
        "xT": xT, "x0": x0,
        "Wk1": f("Wk1"), "Wq1": f("Wq1"), "Wk2": f("Wk2"), "Wq2": f("Wq2"),
        "Wv2": f("Wv2"), "bq1": f("bq1"), "bq2": f("bq2"),
        "bv2row": f("bv2").reshape(1, D),
        "ones": np.ones((1, P), np.float32),
        "onescol": np.ones((P, 1), np.float32),
        "ident": np.eye(P, dtype=np.float32),
    }
    in_maps = [
        {**base, "xTq": np.ascontiguousarray(xT[:, c * SH:(c + 1) * SH])}
        for c in range(C)
    ]

    if "nc" not in _cache:
        _cache["nc"] = _build()
    res = run_bass_kernel_spmd(_cache["nc"], in_maps, list(range(C)))
    return res.results[0]["out"].astype(np.float32)


if __name__ == "__main__":
    d = np.load("/root/problem/inputs.npz")
    out = kernel(**{k: d[k] for k in d.files})
    ref = np.load("/root/problem/ref_out.npy")
    rel = np.abs(out - ref).max() / np.abs(ref).max()
    print("Relative error:", rel)



# revision 39
# speedup vs baseline: 1.2262x; 1.2262x over previous
"""Trainium2 Bass kernel for nn_ModelAttention2Layers (B=8, S=2048, D=512, K=256).

Key structural insight: the reference returns final[0, -1, :] — only batch 0
matters (attention is independent per batch element), so batches 1-7 are dead
compute. Strategy: shard the 2048-query sequence of batch 0 across the 8
cores (256 queries each), with:
  - block 1 fully local per core (xT replicated -> k1T computed redundantly,
    so block 1 needs zero collectives)
  - one AllGather of the {k2T, v2} shards for block 2
  - block 3 flash-style: tiny AllGather of hidden[-1], per-core partial
    softmax/AV over the local 256 keys, one tiny AllReduce of [o|l]
Matmuls run in float32r (full-rate PE, ~11-bit mantissa); softmax statistics,
normalization and reductions in float32. k-projection biases are dropped —
they shift each query's logits by a per-query constant, which softmax
cancels exactly.
"""
import sys

sys.path.insert(0, "/opt/trn_rl_repo")

import numpy as np

S, D, K, P, C = 2048, 512, 256, 128, 8
SH = S // C          # 256 queries/keys per core
ND, NK, NS, NSH = D // P, K // P, S // P, SH // P   # 4, 2, 16, 2
TRN2_NC_BASE = (0, 1, 2, 3, 6, 7, 4, 5)

_cache = {}


def _build():
    import concourse.bass as bass
    import concourse.tile as tile
    from concourse import mybir, bacc
    from bass_rust import add_dep_helper

    F32 = mybir.dt.float32
    F32R = mybir.dt.float32r
    BF16 = mybir.dt.bfloat16
    AF = mybir.ActivationFunctionType
    ts = bass.ts

    nc = bacc.Bacc()

    ins = {}
    for name, shape in [
        ("xT", [D, S]), ("x0", [S, D]), ("xTq", [D, SH]),
        ("Wk1", [D, K]), ("Wq1", [D, K]), ("Wk2", [D, K]), ("Wq2", [D, K]),
        ("Wv2", [D, D]), ("bq1", [K]), ("bq2", [K]), ("bv2row", [1, D]),
        ("ones", [1, P]), ("onescol", [P, 1]), ("ident", [P, P]),
        ("is7", [P, 1]),
    ]:
        ins[name] = nc.dram_tensor(name, shape, F32, kind="ExternalInput")
    out_ext = nc.dram_tensor("out", [D], F32, kind="ExternalOutput")

    GA = NK * P * K + NK * P * D   # gather-A floats per core: k2T + v2 shards

    with tile.TileContext(nc) as tc:
        with tc.tile_pool(name="const", bufs=1) as cw, \
             tc.tile_pool(name="big", bufs=1) as big, \
             tc.tile_pool(name="work", bufs=1) as wk, \
             tc.tile_pool(name="pp", bufs=2) as pp, \
             tc.tile_pool(name="small", bufs=2) as sm, \
             tc.tile_pool(name="stage", bufs=2) as stg, \
             tc.tile_pool(name="ps", bufs=1, space="PSUM") as ps, \
             tc.tile_pool(name="dram", bufs=1, space="DRAM") as dram, \
             tc.tile_pool(name="shdram", bufs=1, space="DRAM") as shd:

            # ---- input loads ----
            # small weights cast-load via gpsimd; bulk tensors (xT, x0) load as
            # f32 on sync HWDGE queues and cast to f32r on DVE (parallel paths,
            # so the first k1T matmul can start within a few us)
            W_r = {}
            for w, ncol in [("Wk1", K), ("Wq1", K)]:
                W_r[w] = cw.tile([P, ND, ncol], F32R, name=f"W_{w}", tag=f"W_{w}")
                nc.gpsimd.dma_start(W_r[w][:], ins[w][:].rearrange("(k p) n -> p k n", p=P))
            xT_r = big.tile([P, ND, S], F32R, tag="XV")
            for k in range(ND):
                st = stg.tile([P, S], F32, tag="stg")
                nc.sync.dma_start(
                    st[:], ins["xT"][:].rearrange("(k2 p) s -> p k2 s", p=P)[:, k, :])
                nc.vector.tensor_copy(xT_r[:, k, :], st[:])
            xTq_r = cw.tile([P, ND, SH], F32R)
            nc.gpsimd.dma_start(xTq_r[:], ins["xTq"][:].rearrange("(k p) j -> p k j", p=P))
            x0_r = cw.tile([P, NS, D], F32R)
            for n4 in range(4):
                st = stg.tile([P, S], F32, tag="stg")
                nc.sync.dma_start(
                    st[:].rearrange("p (n d) -> p n d", n=4),
                    ins["x0"][:].rearrange("(n p) d -> p n d", p=P)[:, 4 * n4:4 * n4 + 4, :])
                nc.vector.tensor_copy(
                    x0_r[:, 4 * n4:4 * n4 + 4, :].rearrange("p n d -> p (n d)"), st[:])
            for w, ncol in [("Wk2", K), ("Wq2", K), ("Wv2", D)]:
                W_r[w] = cw.tile([P, ND, ncol], F32R, name=f"W_{w}", tag=f"W_{w}")
                st = stg.tile([P, ND * ncol], F32, tag="stg", name=f"st_{w}")
                nc.sync.dma_start(
                    st[:].rearrange("p (k n) -> p k n", k=ND),
                    ins[w][:].rearrange("(k p) n -> p k n", p=P))
                nc.vector.tensor_copy(
                    W_r[w][:].rearrange("p k n -> p (k n)"), st[:])
            bq1_sb = cw.tile([P, NK], F32)
            nc.sync.dma_start(bq1_sb[:], ins["bq1"][:].rearrange("(m p) -> p m", p=P))
            bq2_sb = cw.tile([P, NK], F32)
            nc.sync.dma_start(bq2_sb[:], ins["bq2"][:].rearrange("(m p) -> p m", p=P))
            bv2_r = cw.tile([1, D], F32R)
            nc.gpsimd.dma_start(bv2_r[:], ins["bv2row"][:])
            ones_r = cw.tile([1, P], F32R)
            nc.gpsimd.dma_start(ones_r[:], ins["ones"][:])
            ident_r = cw.tile([P, P], F32R)
            nc.gpsimd.dma_start(ident_r[:], ins["ident"][:])
            Wq2f = cw.tile([P, ND, K], F32)
            nc.sync.dma_start(Wq2f[:], ins["Wq2"][:].rearrange("(k p) n -> p k n", p=P))
            onescol_f = cw.tile([P, 1], F32)
            nc.sync.dma_start(onescol_f[:], ins["onescol"][:])
            is7_sb = cw.tile([P, 1], F32)
            nc.sync.dma_start(is7_sb[:], ins["is7"][:])
            ones_f = cw.tile([1, P], F32)
            nc.sync.dma_start(ones_f[:], ins["ones"][:])

            # ---- block 1 projections ----
            # k1T full [K, S], computed redundantly on every core (no bias: softmax-invariant)
            k1T = big.tile([P, NK, S], F32R, tag="kT")
            for m in range(NK):
                for cb in range(S // 512):
                    pm = ps.tile([P, 512], F32, tag="mm")
                    for k in range(ND):
                        nc.tensor.matmul(pm[:], W_r["Wk1"][:, k, ts(m, P)],
                                         xT_r[:, k, ts(cb, 512)],
                                         start=(k == 0), stop=(k == ND - 1))
                    nc.vector.tensor_copy(k1T[:, m, ts(cb, 512)], pm[:])
            # q1T shard [K, SH] with bias bq1
            q1T = wk.tile([P, NK, SH], F32R, tag="qT")
            for m in range(NK):
                pm = ps.tile([P, SH], F32, tag="mm")
                for k in range(ND):
                    nc.tensor.matmul(pm[:], W_r["Wq1"][:, k, ts(m, P)], xTq_r[:, k, :],
                                     start=(k == 0), stop=(k == ND - 1))
                nc.vector.tensor_scalar_add(q1T[:, m, :], pm[:], bq1_sb[:, m:m + 1])

            def attention(qT, kT_full, V_full, out_dst, pt_dtype):
                """out_dst[:, qm, :] = softmax(q.k^T) @ V for this core's 256 queries."""
                for qm in range(NSH):
                    sc = ps.tile([P, 4, 512], F32, tag="sc")
                    for ks in range(4):
                        for dm in range(NK):
                            nc.tensor.matmul(sc[:, ks, :], qT[:, dm, ts(qm, P)],
                                             kT_full[:, dm, ts(ks, 512)],
                                             start=(dm == 0), stop=(dm == NK - 1))
                    mx = sm.tile([P, 1], F32, tag="mx")
                    nc.vector.reduce_max(mx[:], sc[:], axis=mybir.AxisListType.XY)
                    nm = sm.tile([P, 1], F32, tag="nm")
                    nc.vector.tensor_scalar_mul(nm[:], mx[:], -1.0)
                    Pt = pp.tile([P, S], F32R, tag="P")
                    lsum = sm.tile([P, 4], F32, tag="lsum")
                    for ks in range(4):
                        nc.scalar.activation(Pt[:, ts(ks, 512)], sc[:, ks, :], AF.Exp,
                                             bias=nm[:], accum_out=lsum[:, ks:ks + 1])
                    l = sm.tile([P, 1], F32, tag="l")
                    nc.vector.reduce_sum(l[:], lsum[:], axis=mybir.AxisListType.X)
                    rl = sm.tile([P, 1], F32, tag="rl")
                    nc.vector.reciprocal(rl[:], l[:])
                    PT = pp.tile([P, NS, P], pt_dtype, tag="PT")
                    for n in range(NS):
                        tp = ps.tile([P, P], F32R, tag="tp")
                        nc.tensor.transpose(tp[:], Pt[:, ts(n, P)], ident_r[:])
                        nc.vector.tensor_copy(PT[:, n, :], tp[:])
                    av = ps.tile([P, D], F32, tag="mm")
                    for n in range(NS):
                        nc.tensor.matmul(av[:], PT[:, n, :], V_full[:, n, :],
                                         start=(n == 0), stop=(n == NS - 1))
                    nc.scalar.activation(out_dst[:, qm, :], av[:], AF.Copy, scale=rl[:])

            out1 = wk.tile([P, NSH, D], F32R, tag="H")
            attention(q1T, k1T, x0_r, out1, F32R)

            def transpose_rows(src, ncols_chunks):
                """src [P, NSH, D] -> dst [P, ND, SH] (row-major shard transposed)."""
                dst = wk.tile([P, ND, SH], F32R, tag="HT")
                for qm in reversed(range(NSH)):
                    for dm in range(ND):
                        tp = ps.tile([P, P], F32R, tag="tp")
                        nc.tensor.transpose(tp[:], src[:, qm, ts(dm, P)], ident_r[:])
                        nc.vector.tensor_copy(dst[:, dm, ts(qm, P)], tp[:])
                return dst

            out1T = transpose_rows(out1, ND)

            # ---- block 2 shard projections ----
            k2T = wk.tile([P, NK, SH], BF16, tag="kv_k")
            for m in range(NK):
                pm = ps.tile([P, SH], F32, tag="mm")
                for k in range(ND):
                    nc.tensor.matmul(pm[:], W_r["Wk2"][:, k, ts(m, P)], out1T[:, k, :],
                                     start=(k == 0), stop=(k == ND - 1))
                nc.vector.tensor_copy(k2T[:, m, :], pm[:])
            # gather 1 (k2T) fires while q2T/v2 are still being computed
            gk_in = dram.tile([NK * P * SH], BF16)
            nc.sync.dma_start(
                gk_in[:].rearrange("(m p j) -> p m j", m=NK, p=P), k2T[:])
            gk_out = shd.tile([C, NK * P * SH], BF16, addr_space="Shared")
            nc.gpsimd.collective_compute(
                "AllGather", mybir.AluOpType.bypass,
                replica_groups=[list(range(C))],
                ins=[gk_in.opt()], outs=[gk_out.opt()],
            )
            k2T_full = big.tile([P, NK, S], BF16, tag="kT")
            for m in range(NK):
                nc.sync.dma_start(
                    k2T_full[:, m, :].rearrange("p (c j) -> p c j", c=C),
                    gk_out[:, m * P * SH:(m + 1) * P * SH].rearrange(
                        "c (p j) -> p c j", p=P))
            q2T = wk.tile([P, NK, SH], BF16, tag="qT")
            for m in range(NK):
                pm = ps.tile([P, SH], F32, tag="mm")
                for k in range(ND):
                    nc.tensor.matmul(pm[:], W_r["Wq2"][:, k, ts(m, P)], out1T[:, k, :],
                                     start=(k == 0), stop=(k == ND - 1))
                nc.vector.tensor_scalar_add(q2T[:, m, :], pm[:], bq2_sb[:, m:m + 1])

            def vproj_norm(hT, out_dtype):
                """v = normalize_rows(h @ Wv2 + bv2) for this core's 256 rows."""
                v_sb = wk.tile([P, NSH, D], out_dtype, tag="kv_v")
                for r in range(NSH):
                    pm = ps.tile([P, D], F32, tag="mm")
                    for k in range(ND):
                        nc.tensor.matmul(pm[:], hT[:, k, ts(r, P)], W_r["Wv2"][:, k, :],
                                         start=(k == 0), stop=False)
                    nc.tensor.matmul(pm[:], ones_r[:], bv2_r[:], start=False, stop=True)
                    scr = sm.tile([P, D], F32, tag="scr")
                    ssum = sm.tile([P, 1], F32, tag="ssum")
                    nc.scalar.activation(scr[:], pm[:], AF.Square, accum_out=ssum[:])
                    nrm = sm.tile([P, 1], F32, tag="nrm")
                    nc.scalar.sqrt(nrm[:], ssum[:])
                    rn = sm.tile([P, 1], F32, tag="rn")
                    nc.vector.reciprocal(rn[:], nrm[:])
                    nc.scalar.activation(v_sb[:, r, :], pm[:], AF.Copy, scale=rn[:])
                return v_sb

            v2 = vproj_norm(out1T, BF16)

            gv_in = dram.tile([NSH * P * D], BF16)
            nc.sync.dma_start(
                gv_in[:].rearrange("(r p d) -> p r d", r=NSH, p=P), v2[:])
            gv_out = shd.tile([C, NSH * P * D], BF16, addr_space="Shared")
            nc.gpsimd.collective_compute(
                "AllGather", mybir.AluOpType.bypass,
                replica_groups=[list(range(C))],
                ins=[gv_in.opt()], outs=[gv_out.opt()],
            )
            v2_full = big.tile([P, NS, D], BF16, tag="XV")
            for r in range(NSH):
                off = r * P * D
                nc.sync.dma_start(
                    v2_full[:].rearrange("p (c r) d -> p c r d", c=C)[:, :, r, :],
                    gv_out[:, off:off + P * D].rearrange("c (p d) -> p c d", p=P))

            # ---- block 2 attention ----
            hidden = wk.tile([P, NSH, D], F32R, tag="H")
            attention(q2T, k2T_full, v2_full, hidden, BF16)
            hT = transpose_rows(hidden, ND)

            # hidden[-1] lives on core 7; every core sends its last local row
            # scaled by is7 (1.0 only on core 7) to XOR-slot k of every peer,
            # and the receivers sum all 8 slots -> exactly core 7's row.
            rsem_hl = nc.alloc_semaphore("rsem_hl")
            rsem_ol = nc.alloc_semaphore("rsem_ol")
            lsem = nc.alloc_semaphore("lsem")
            gates = []
            hl_c = wk.tile([P, ND], F32, tag="hl_c")
            nc.vector.tensor_scalar_mul(hl_c[:], hT[:, :, SH - 1], is7_sb[:])
            hlg = wk.tile([P, C, ND], F32, tag="hlg")
            nc.vector.tensor_copy(hlg[:, 0, :], hl_c[:])
            for k in range(1, C):
                rd = [None] * C
                rd[k] = (0, TRN2_NC_BASE[k])
                nc.gpsimd.remote_dma_broadcast(
                    hlg[:, k, :], hl_c[:],
                    remote_sem=rsem_hl, local_sem=lsem, rdests=rd)
            trig_hl = nc.gpsimd.trigger_dma(count=None)
            gate_hl = nc.gpsimd.nop(nofuse=True, hint="gate_hl")
            add_dep_helper(gate_hl.ins, trig_hl.ins, sync=False, reason="order")
            gates.append((gate_hl, rsem_hl, 7 * 2))
            hl_r = sm.tile([P, ND], F32, tag="hl")
            hsum = nc.vector.reduce_sum(hl_r[:], hlg[:].rearrange("p c d -> p d c"),
                                        axis=mybir.AxisListType.X)
            add_dep_helper(hsum.ins, gate_hl.ins, sync=True, reason="hl gather")

            # ---- block 3 (flash-style partials over this core's 256 keys) ----
            k3T = wk.tile([P, NK, SH], F32, tag="kv_k")
            for m in range(NK):
                pm = ps.tile([P, SH], F32, tag="mm")
                for k in range(ND):
                    nc.tensor.matmul(pm[:], W_r["Wk2"][:, k, ts(m, P)], hT[:, k, :],
                                     start=(k == 0), stop=(k == ND - 1))
                nc.vector.tensor_copy(k3T[:, m, :], pm[:])
            v3 = vproj_norm(hT, F32)

            # q3 = Wq2^T @ h_last + bq2
            q3 = sm.tile([P, NK], F32, tag="q3")
            for fm in range(NK):
                pm = ps.tile([P, 1], F32, tag="mm")
                for dm in range(ND):
                    nc.tensor.matmul(pm[:], Wq2f[:, dm, ts(fm, P)], hl_r[:, dm:dm + 1],
                                     start=(dm == 0), stop=(dm == ND - 1))
                nc.vector.tensor_scalar_add(q3[:, fm:fm + 1], pm[:], bq2_sb[:, fm:fm + 1])

            # s3 (scores for my 256 keys; |s3| <= ~4 so exp needs no max shift)
            s3p = ps.tile([P, NSH], F32, tag="tp")
            for n in range(NSH):
                for fm in range(NK):
                    nc.tensor.matmul(s3p[:, n:n + 1], k3T[:, fm, ts(n, P)], q3[:, fm:fm + 1],
                                     start=(fm == 0), stop=(fm == NK - 1))
            p3 = sm.tile([P, NSH], F32, tag="p3")
            nc.scalar.activation(p3[:], s3p[:], AF.Exp)

            # partial numerator oT [128, ND] (d on partitions) + replicated l,
            # gathered via XOR-slot remote broadcasts and summed on every core
            ol_ps = ps.tile([P, ND + 1], F32, tag="mm")
            for dm in range(ND):
                for n in range(NSH):
                    nc.tensor.matmul(ol_ps[:, dm:dm + 1], v3[:, n, ts(dm, P)],
                                     p3[:, n:n + 1],
                                     start=(n == 0), stop=(n == NSH - 1))
            l3p = ps.tile([1, 1], F32, tag="tp")
            for n in range(NSH):
                nc.tensor.matmul(l3p[:], p3[:, n:n + 1], onescol_f[:],
                                 start=(n == 0), stop=(n == NSH - 1))
            l3f = sm.tile([1, 1], F32, tag="l3f")
            nc.vector.tensor_copy(l3f[:], l3p[:])
            nc.tensor.matmul(ol_ps[:, ND:ND + 1], ones_f[:], l3f[:],
                             start=True, stop=True)
            ol = wk.tile([P, ND + 1], F32, tag="ol")
            nc.vector.tensor_copy(ol[:], ol_ps[:])

            olg = wk.tile([P, C, ND + 1], F32, tag="olg")
            nc.vector.tensor_copy(olg[:, 0, :], ol[:])
            for k in range(1, C):
                rd = [None] * C
                rd[k] = (0, TRN2_NC_BASE[k])
                nc.gpsimd.remote_dma_broadcast(
                    olg[:, k, :], ol[:],
                    remote_sem=rsem_ol, local_sem=lsem, rdests=rd)
            trig_ol = nc.gpsimd.trigger_dma(count=None)
            gate_ol = nc.gpsimd.nop(nofuse=True, hint="gate_ol")
            add_dep_helper(gate_ol.ins, trig_ol.ins, sync=False, reason="order")
            gates.append((gate_ol, rsem_ol, 7 * 2))
            tot = wk.tile([P, ND + 1], F32, tag="tot")
            osum = nc.vector.reduce_sum(tot[:], olg[:].rearrange("p c e -> p e c"),
                                        axis=mybir.AxisListType.X)
            add_dep_helper(osum.ins, gate_ol.ins, sync=True, reason="ol gather")
            rl3 = sm.tile([P, 1], F32, tag="rl3")
            nc.vector.reciprocal(rl3[:], tot[:, ND:ND + 1])
            fin = wk.tile([P, ND], F32, tag="fin")
            nc.vector.tensor_scalar_mul(fin[:], tot[:, 0:ND], rl3[:])
            nc.sync.dma_start(out_ext[:].rearrange("(k p) -> p k", p=P), fin[:])

    for gate, sem, target in gates:
        gate.wait_op(sem, target, "sem-ge")
    nc.finalize()
    return nc


def kernel(**inputs):
    from concourse.bass_utils import run_bass_kernel_spmd

    f = lambda k: np.ascontiguousarray(np.asarray(inputs[k], dtype=np.float32))
    x0 = f("x")[0]                       # [S, D]; batches 1..7 are dead
    xT = np.ascontiguousarray(x0.T)      # [D, S]
    base = {
        "xT": xT, "x0": x0,
        "Wk1": f("Wk1"), "Wq1": f("Wq1"), "Wk2": f("Wk2"), "Wq2": f("Wq2"),
        "Wv2": f("Wv2"), "bq1": f("bq1"), "bq2": f("bq2"),
        "bv2row": f("bv2").reshape(1, D),
        "ones": np.ones((1, P), np.float32),
        "onescol": np.ones((P, 1), np.float32),
        "ident": np.eye(P, dtype=np.float32),
    }
    in_maps = [
        {**base,
         "xTq": np.ascontiguousarray(xT[:, c * SH:(c + 1) * SH]),
         "is7": np.full((P, 1), 1.0 if c == C - 1 else 0.0, np.float32)}
        for c in range(C)
    ]

    if "nc" not in _cache:
        _cache["nc"] = _build()
    res = run_bass_kernel_spmd(_cache["nc"], in_maps, list(range(C)))
    return res.results[0]["out"].astype(np.float32)


if __name__ == "__main__":
    d = np.load("/root/problem/inputs.npz")
    out = kernel(**{k: d[k] for k in d.files})
    ref = np.load("/root/problem/ref_out.npy")
    rel = np.abs(out - ref).max() / np.abs(ref).max()
    print("Relative error:", rel)



# revision 44
# speedup vs baseline: 1.8862x; 1.5382x over previous
"""Trainium2 Bass kernel for nn_ModelAttention2Layers (B=8, S=2048, D=512, K=256).

Key structural insight: the reference returns final[0, -1, :] — only batch 0
matters (attention is independent per batch element), so batches 1-7 are dead
compute. Strategy: shard the 2048-query sequence of batch 0 across the 8
cores (256 queries each), with:
  - block 1 fully local per core (xT replicated -> k1T computed redundantly,
    so block 1 needs zero collectives)
  - one AllGather of the {k2T, v2} shards for block 2
  - block 3 flash-style: tiny AllGather of hidden[-1], per-core partial
    softmax/AV over the local 256 keys, one tiny AllReduce of [o|l]
Matmuls run in float32r (full-rate PE, ~11-bit mantissa); softmax statistics,
normalization and reductions in float32. k-projection biases are dropped —
they shift each query's logits by a per-query constant, which softmax
cancels exactly.
"""
import sys

sys.path.insert(0, "/opt/trn_rl_repo")

import numpy as np

S, D, K, P, C = 2048, 512, 256, 128, 8
SH = S // C          # 256 queries/keys per core
ND, NK, NS, NSH = D // P, K // P, S // P, SH // P   # 4, 2, 16, 2
TRN2_NC_BASE = (0, 1, 2, 3, 6, 7, 4, 5)

_cache = {}


def _build():
    import concourse.bass as bass
    import concourse.tile as tile
    from concourse import mybir, bacc
    from bass_rust import add_dep_helper

    F32 = mybir.dt.float32
    F32R = mybir.dt.float32r
    BF16 = mybir.dt.bfloat16
    AF = mybir.ActivationFunctionType
    ts = bass.ts

    nc = bacc.Bacc()

    ins = {}
    for name, shape in [
        ("xT", [D, S]), ("x0", [S, D]), ("xTq", [D, SH]),
        ("Wk1", [D, K]), ("Wq1", [D, K]), ("Wk2", [D, K]), ("Wq2", [D, K]),
        ("Wv2", [D, D]), ("bq1", [K]), ("bq2", [K]), ("bv2row", [1, D]),
        ("ones", [1, P]), ("onescol", [P, 1]), ("ident", [P, P]),
        ("is7", [P, 1]),
    ]:
        ins[name] = nc.dram_tensor(name, shape, F32, kind="ExternalInput")
    out_ext = nc.dram_tensor("out", [D], F32, kind="ExternalOutput")

    GA = NK * P * K + NK * P * D   # gather-A floats per core: k2T + v2 shards

    with tile.TileContext(nc) as tc:
        lsem2 = nc.alloc_semaphore("lsem2")
        with tc.tile_pool(name="const", bufs=1) as cw, \
             tc.tile_pool(name="big", bufs=1) as big, \
             tc.tile_pool(name="work", bufs=1) as wk, \
             tc.tile_pool(name="pp", bufs=2) as pp, \
             tc.tile_pool(name="small", bufs=2) as sm, \
             tc.tile_pool(name="stage", bufs=1) as stg, \
             tc.tile_pool(name="ps", bufs=1, space="PSUM") as ps, \
             tc.tile_pool(name="dram", bufs=1, space="DRAM") as dram, \
             tc.tile_pool(name="shdram", bufs=1, space="DRAM") as shd:

            # ---- input loads ----
            # small weights cast-load via gpsimd; bulk tensors (xT, x0) load as
            # f32 on sync HWDGE queues and cast to f32r on DVE (parallel paths,
            # so the first k1T matmul can start within a few us)
            W_r = {}
            for w, ncol in [("Wk1", K), ("Wq1", K)]:
                W_r[w] = cw.tile([P, ND, ncol], F32R, name=f"W_{w}", tag=f"W_{w}")
                nc.gpsimd.dma_start(W_r[w][:], ins[w][:].rearrange("(k p) n -> p k n", p=P))
            xT_r = big.tile([P, ND, S], F32R, tag="XV")
            for k in range(ND):
                st = stg.tile([P, S], F32, tag="stg")
                nc.sync.dma_start(
                    st[:], ins["xT"][:].rearrange("(k2 p) s -> p k2 s", p=P)[:, k, :])
                nc.vector.tensor_copy(xT_r[:, k, :], st[:])
            xTq_r = cw.tile([P, ND, SH], F32R)
            nc.gpsimd.dma_start(xTq_r[:], ins["xTq"][:].rearrange("(k p) j -> p k j", p=P))
            x0_r = cw.tile([P, NS, D], F32R)
            for n4 in range(4):
                st = stg.tile([P, S], F32, tag="stg")
                nc.sync.dma_start(
                    st[:].rearrange("p (n d) -> p n d", n=4),
                    ins["x0"][:].rearrange("(n p) d -> p n d", p=P)[:, 4 * n4:4 * n4 + 4, :])
                nc.vector.tensor_copy(
                    x0_r[:, 4 * n4:4 * n4 + 4, :].rearrange("p n d -> p (n d)"), st[:])
            for w, ncol in [("Wk2", K), ("Wq2", K), ("Wv2", D)]:
                W_r[w] = cw.tile([P, ND, ncol], F32R, name=f"W_{w}", tag=f"W_{w}")
                st = stg.tile([P, ND * ncol], F32, tag="stg", name=f"st_{w}")
                nc.sync.dma_start(
                    st[:].rearrange("p (k n) -> p k n", k=ND),
                    ins[w][:].rearrange("(k p) n -> p k n", p=P))
                nc.vector.tensor_copy(
                    W_r[w][:].rearrange("p k n -> p (k n)"), st[:])
            bq1_sb = cw.tile([P, NK], F32)
            nc.sync.dma_start(bq1_sb[:], ins["bq1"][:].rearrange("(m p) -> p m", p=P))
            bq2_sb = cw.tile([P, NK], F32)
            nc.sync.dma_start(bq2_sb[:], ins["bq2"][:].rearrange("(m p) -> p m", p=P))
            bv2_r = cw.tile([1, D], F32R)
            nc.gpsimd.dma_start(bv2_r[:], ins["bv2row"][:])
            ones_r = cw.tile([1, P], F32R)
            nc.gpsimd.dma_start(ones_r[:], ins["ones"][:])
            ident_r = cw.tile([P, P], F32R)
            nc.gpsimd.dma_start(ident_r[:], ins["ident"][:])
            onescol_f = cw.tile([P, 1], F32)
            nc.sync.dma_start(onescol_f[:], ins["onescol"][:])
            is7_sb = cw.tile([P, 1], F32)
            nc.sync.dma_start(is7_sb[:], ins["is7"][:])
            ones_f = cw.tile([1, P], F32)
            nc.sync.dma_start(ones_f[:], ins["ones"][:])

            # ---- block 1 projections ----
            # k1T full [K, S], computed redundantly on every core (no bias: softmax-invariant)
            k1T = big.tile([P, NK, S], F32R, tag="kT")
            for m in range(NK):
                for cb in range(S // 512):
                    pm = ps.tile([P, 512], F32, tag="mm")
                    for k in range(ND):
                        nc.tensor.matmul(pm[:], W_r["Wk1"][:, k, ts(m, P)],
                                         xT_r[:, k, ts(cb, 512)],
                                         start=(k == 0), stop=(k == ND - 1))
                    nc.vector.tensor_copy(k1T[:, m, ts(cb, 512)], pm[:])
            # q1T shard [K, SH] with bias bq1
            q1T = wk.tile([P, NK, SH], F32R, tag="qT")
            for m in range(NK):
                pm = ps.tile([P, SH], F32, tag="mm")
                for k in range(ND):
                    nc.tensor.matmul(pm[:], W_r["Wq1"][:, k, ts(m, P)], xTq_r[:, k, :],
                                     start=(k == 0), stop=(k == ND - 1))
                nc.vector.tensor_scalar_add(q1T[:, m, :], pm[:], bq1_sb[:, m:m + 1])

            def attention(qT, kT_full, V_full, out_dst, pt_dtype,
                          slot=False, score_gate=None, av_gate=None):
                """out_dst[:, qm, :] = softmax(q.k^T) @ V for this core's 256
                queries. slot=True reads keys/values in XOR-slot-major order
                (consistent between k and v, so softmax output is exact)."""
                for qm in range(NSH):
                    sc = ps.tile([P, 4, 512], F32, tag="sc")
                    for ks in range(4):
                        if slot:
                            for jj2 in range(2):
                                j = 2 * ks + jj2
                                for dm in range(NK):
                                    mm = nc.tensor.matmul(
                                        sc[:, ks, jj2 * SH:(jj2 + 1) * SH],
                                        qT[:, dm, ts(qm, P)], kT_full[:, j, dm, :],
                                        start=(dm == 0), stop=(dm == NK - 1))
                                    if score_gate is not None:
                                        add_dep_helper(mm.ins, score_gate.ins,
                                                       sync=True, reason="gk")
                        else:
                            for dm in range(NK):
                                nc.tensor.matmul(sc[:, ks, :], qT[:, dm, ts(qm, P)],
                                                 kT_full[:, dm, ts(ks, 512)],
                                                 start=(dm == 0), stop=(dm == NK - 1))
                    mx = sm.tile([P, 1], F32, tag="mx")
                    nc.vector.reduce_max(mx[:], sc[:], axis=mybir.AxisListType.XY)
                    nm = sm.tile([P, 1], F32, tag="nm")
                    nc.vector.tensor_scalar_mul(nm[:], mx[:], -1.0)
                    Pt = pp.tile([P, S], F32R, tag="P")
                    lsum = sm.tile([P, 4], F32, tag="lsum")
                    for ks in range(4):
                        nc.scalar.activation(Pt[:, ts(ks, 512)], sc[:, ks, :], AF.Exp,
                                             bias=nm[:], accum_out=lsum[:, ks:ks + 1])
                    l = sm.tile([P, 1], F32, tag="l")
                    nc.vector.reduce_sum(l[:], lsum[:], axis=mybir.AxisListType.X)
                    rl = sm.tile([P, 1], F32, tag="rl")
                    nc.vector.reciprocal(rl[:], l[:])
                    PT = pp.tile([P, NS, P], pt_dtype, tag="PT")
                    for n in range(NS):
                        tp = ps.tile([P, P], F32R, tag="tp")
                        nc.tensor.transpose(tp[:], Pt[:, ts(n, P)], ident_r[:])
                        nc.vector.tensor_copy(PT[:, n, :], tp[:])
                    av = ps.tile([P, D], F32, tag="mm")
                    for n in range(NS):
                        vb = (V_full[:, n // NSH, n % NSH, :] if slot
                              else V_full[:, n, :])
                        mm = nc.tensor.matmul(av[:], PT[:, n, :], vb,
                                              start=(n == 0), stop=(n == NS - 1))
                        if av_gate is not None:
                            add_dep_helper(mm.ins, av_gate.ins, sync=True,
                                           reason="gv")
                    nc.scalar.activation(out_dst[:, qm, :], av[:], AF.Copy, scale=rl[:])

            out1 = wk.tile([P, NSH, D], F32R, tag="H")
            attention(q1T, k1T, x0_r, out1, F32R)

            def transpose_rows(src, ncols_chunks):
                """src [P, NSH, D] -> dst [P, ND, SH] (row-major shard transposed)."""
                dst = wk.tile([P, ND, SH], F32R, tag="HT")
                for qm in reversed(range(NSH)):
                    for dm in range(ND):
                        tp = ps.tile([P, P], F32R, tag="tp")
                        nc.tensor.transpose(tp[:], src[:, qm, ts(dm, P)], ident_r[:])
                        nc.vector.tensor_copy(dst[:, dm, ts(qm, P)], tp[:])
                return dst

            out1T = transpose_rows(out1, ND)

            # ---- block 2 shard projections ----
            k2T = wk.tile([P, NK, SH], BF16, tag="kv_k")
            for m in range(NK):
                pm = ps.tile([P, SH], F32, tag="mm")
                for k in range(ND):
                    nc.tensor.matmul(pm[:], W_r["Wk2"][:, k, ts(m, P)], out1T[:, k, :],
                                     start=(k == 0), stop=(k == ND - 1))
                nc.vector.tensor_copy(k2T[:, m, :], pm[:])
            # gather 1 (k2T): XOR-slot remote broadcasts; slot k on core r holds
            # core (r^k)'s keys. v2 uses the same slot map, so attention2 sees a
            # consistently permuted key order (softmax is permutation-invariant).
            rsem_k2 = nc.alloc_semaphore("rsem_k2")
            rsem_v2 = nc.alloc_semaphore("rsem_v2")
            k2_full = big.tile([P, C, NK, SH], BF16, tag="k2f")
            nc.vector.tensor_copy(k2_full[:, 0], k2T[:])
            for k in range(1, C):
                rd = [None] * C
                rd[k] = (0, TRN2_NC_BASE[k])
                nc.gpsimd.remote_dma_broadcast(
                    k2_full[:, k], k2T[:],
                    remote_sem=rsem_k2, local_sem=lsem2, rdests=rd)
            trig_k2 = nc.gpsimd.trigger_dma(count=None)
            q2T = wk.tile([P, NK, SH], BF16, tag="qT")
            for m in range(NK):
                pm = ps.tile([P, SH], F32, tag="mm")
                for k in range(ND):
                    nc.tensor.matmul(pm[:], W_r["Wq2"][:, k, ts(m, P)], out1T[:, k, :],
                                     start=(k == 0), stop=(k == ND - 1))
                nc.vector.tensor_scalar_add(q2T[:, m, :], pm[:], bq2_sb[:, m:m + 1])

            def vproj_norm(hT, out_dtype, tag="kv_v"):
                """v = normalize_rows(h @ Wv2 + bv2) for this core's 256 rows."""
                v_sb = wk.tile([P, NSH, D], out_dtype, tag=tag)
                for r in range(NSH):
                    pm = ps.tile([P, D], F32, tag="mm")
                    for k in range(ND):
                        nc.tensor.matmul(pm[:], hT[:, k, ts(r, P)], W_r["Wv2"][:, k, :],
                                         start=(k == 0), stop=False)
                    nc.tensor.matmul(pm[:], ones_r[:], bv2_r[:], start=False, stop=True)
                    scr = sm.tile([P, D], F32, tag="scr")
                    ssum = sm.tile([P, 1], F32, tag="ssum")
                    nc.scalar.activation(scr[:], pm[:], AF.Square, accum_out=ssum[:])
                    nrm = sm.tile([P, 1], F32, tag="nrm")
                    nc.scalar.sqrt(nrm[:], ssum[:])
                    rn = sm.tile([P, 1], F32, tag="rn")
                    nc.vector.reciprocal(rn[:], nrm[:])
                    nc.scalar.activation(v_sb[:, r, :], pm[:], AF.Copy, scale=rn[:])
                return v_sb

            v2 = vproj_norm(out1T, BF16)

            v2_full = big.tile([P, C, NSH, D], BF16, tag="v2f")
            nc.vector.tensor_copy(v2_full[:, 0], v2[:])
            for k in range(1, C):
                rd = [None] * C
                rd[k] = (0, TRN2_NC_BASE[k])
                nc.gpsimd.remote_dma_broadcast(
                    v2_full[:, k], v2[:],
                    remote_sem=rsem_v2, local_sem=lsem2, rdests=rd)
            trig_v2 = nc.gpsimd.trigger_dma(count=None)
            gate_k2 = nc.gpsimd.nop(nofuse=True, hint="gate_k2")
            add_dep_helper(gate_k2.ins, trig_k2.ins, sync=False, reason="order")
            add_dep_helper(gate_k2.ins, trig_v2.ins, sync=False, reason="order")
            gate_v2 = nc.gpsimd.nop(nofuse=True, hint="gate_v2")
            add_dep_helper(gate_v2.ins, gate_k2.ins, sync=False, reason="order")
            gates2 = [(gate_k2, rsem_k2, 7 * 2), (gate_v2, rsem_v2, 7 * 2)]

            # ---- block 2 attention ----
            hidden = wk.tile([P, NSH, D], F32R, tag="H")
            attention(q2T, k2_full, v2_full, hidden, BF16,
                      slot=True, score_gate=gate_k2, av_gate=gate_v2)
            hT = transpose_rows(hidden, ND)

            # hidden[-1] lives on core 7; every core sends its last local row
            # scaled by is7 (1.0 only on core 7) to XOR-slot k of every peer,
            # and the receivers sum all 8 slots -> exactly core 7's row.
            rsem_hl = nc.alloc_semaphore("rsem_hl")
            rsem_ol = nc.alloc_semaphore("rsem_ol")
            lsem = nc.alloc_semaphore("lsem")
            gates = gates2
            hl_c = wk.tile([P, ND], F32, tag="hl_c")
            nc.vector.tensor_scalar_mul(hl_c[:], hT[:, :, SH - 1], is7_sb[:])
            hlg = wk.tile([P, C, ND], F32, tag="hlg")
            nc.vector.tensor_copy(hlg[:, 0, :], hl_c[:])
            for k in range(1, C):
                rd = [None] * C
                rd[k] = (0, TRN2_NC_BASE[k])
                nc.gpsimd.remote_dma_broadcast(
                    hlg[:, k, :], hl_c[:],
                    remote_sem=rsem_hl, local_sem=lsem, rdests=rd)
            trig_hl = nc.gpsimd.trigger_dma(count=None)
            gate_hl = nc.gpsimd.nop(nofuse=True, hint="gate_hl")
            add_dep_helper(gate_hl.ins, trig_hl.ins, sync=False, reason="order")
            gates.append((gate_hl, rsem_hl, 7 * 2))
            hl_r = sm.tile([P, ND], F32, tag="hl")
            hsum = nc.vector.reduce_sum(hl_r[:], hlg[:].rearrange("p c d -> p d c"),
                                        axis=mybir.AxisListType.X)
            add_dep_helper(hsum.ins, gate_hl.ins, sync=True, reason="hl gather")

            # ---- block 3 (flash-style partials over this core's 256 keys) ----
            k3T = wk.tile([P, NK, SH], F32, tag="k3f")
            for m in range(NK):
                pm = ps.tile([P, SH], F32, tag="mm")
                for k in range(ND):
                    nc.tensor.matmul(pm[:], W_r["Wk2"][:, k, ts(m, P)], hT[:, k, :],
                                     start=(k == 0), stop=(k == ND - 1))
                nc.vector.tensor_copy(k3T[:, m, :], pm[:])
            v3 = vproj_norm(hT, F32, tag="v3f")

            # q3 = Wq2^T @ h_last + bq2
            q3 = sm.tile([P, NK], F32, tag="q3")
            for fm in range(NK):
                pm = ps.tile([P, 1], F32, tag="mm")
                for dm in range(ND):
                    nc.tensor.matmul(pm[:], W_r["Wq2"][:, dm, ts(fm, P)].bitcast(F32), hl_r[:, dm:dm + 1],
                                     start=(dm == 0), stop=(dm == ND - 1))
                nc.vector.tensor_scalar_add(q3[:, fm:fm + 1], pm[:], bq2_sb[:, fm:fm + 1])

            # s3 (scores for my 256 keys; |s3| <= ~4 so exp needs no max shift)
            s3p = ps.tile([P, NSH], F32, tag="tp")
            for n in range(NSH):
                for fm in range(NK):
                    nc.tensor.matmul(s3p[:, n:n + 1], k3T[:, fm, ts(n, P)], q3[:, fm:fm + 1],
                                     start=(fm == 0), stop=(fm == NK - 1))
            p3 = sm.tile([P, NSH], F32, tag="p3")
            nc.scalar.activation(p3[:], s3p[:], AF.Exp)

            # partial numerator oT [128, ND] (d on partitions) + replicated l,
            # gathered via XOR-slot remote broadcasts and summed on every core
            ol_ps = ps.tile([P, ND + 1], F32, tag="mm")
            for dm in range(ND):
                for n in range(NSH):
                    nc.tensor.matmul(ol_ps[:, dm:dm + 1], v3[:, n, ts(dm, P)],
                                     p3[:, n:n + 1],
                                     start=(n == 0), stop=(n == NSH - 1))
            l3p = ps.tile([1, 1], F32, tag="tp")
            for n in range(NSH):
                nc.tensor.matmul(l3p[:], p3[:, n:n + 1], onescol_f[:],
                                 start=(n == 0), stop=(n == NSH - 1))
            l3f = sm.tile([1, 1], F32, tag="l3f")
            nc.vector.tensor_copy(l3f[:], l3p[:])
            nc.tensor.matmul(ol_ps[:, ND:ND + 1], ones_f[:], l3f[:],
                             start=True, stop=True)
            ol = wk.tile([P, ND + 1], F32, tag="ol")
            nc.vector.tensor_copy(ol[:], ol_ps[:])

            olg = wk.tile([P, C, ND + 1], F32, tag="olg")
            nc.vector.tensor_copy(olg[:, 0, :], ol[:])
            for k in range(1, C):
                rd = [None] * C
                rd[k] = (0, TRN2_NC_BASE[k])
                nc.gpsimd.remote_dma_broadcast(
                    olg[:, k, :], ol[:],
                    remote_sem=rsem_ol, local_sem=lsem, rdests=rd)
            trig_ol = nc.gpsimd.trigger_dma(count=None)
            gate_ol = nc.gpsimd.nop(nofuse=True, hint="gate_ol")
            add_dep_helper(gate_ol.ins, trig_ol.ins, sync=False, reason="order")
            gates.append((gate_ol, rsem_ol, 7 * 2))
            tot = wk.tile([P, ND + 1], F32, tag="tot")
            osum = nc.vector.reduce_sum(tot[:], olg[:].rearrange("p c e -> p e c"),
                                        axis=mybir.AxisListType.X)
            add_dep_helper(osum.ins, gate_ol.ins, sync=True, reason="ol gather")
            rl3 = sm.tile([P, 1], F32, tag="rl3")
            nc.vector.reciprocal(rl3[:], tot[:, ND:ND + 1])
            fin = wk.tile([P, ND], F32, tag="fin")
            nc.vector.tensor_scalar_mul(fin[:], tot[:, 0:ND], rl3[:])
            nc.sync.dma_start(out_ext[:].rearrange("(k p) -> p k", p=P), fin[:])

    for gate, sem, target in gates:
        gate.wait_op(sem, target, "sem-ge")
    nc.finalize()
    return nc


def kernel(**inputs):
    from concourse.bass_utils import run_bass_kernel_spmd

    f = lambda k: np.ascontiguousarray(np.asarray(inputs[k], dtype=np.float32))
    x0 = f("x")[0]                       # [S, D]; batches 1..7 are dead
    xT = np.ascontiguousarray(x0.T)      # [D, S]
    base = {
        "xT": xT, "x0": x0,
        "Wk1": f("Wk1"), "Wq1": f("Wq1"), "Wk2": f("Wk2"), "Wq2": f("Wq2"),
        "Wv2": f("Wv2"), "bq1": f("bq1"), "bq2": f("bq2"),
        "bv2row": f("bv2").reshape(1, D),
        "ones": np.ones((1, P), np.float32),
        "onescol": np.ones((P, 1), np.float32),
        "ident": np.eye(P, dtype=np.float32),
    }
    in_maps = [
        {**base,
         "xTq": np.ascontiguousarray(xT[:, c * SH:(c + 1) * SH]),
         "is7": np.full((P, 1), 1.0 if c == C - 1 else 0.0, np.float32)}
        for c in range(C)
    ]

    if "nc" not in _cache:
        _cache["nc"] = _build()
    res = run_bass_kernel_spmd(_cache["nc"], in_maps, list(range(C)))
    return res.results[0]["out"].astype(np.float32)


if __name__ == "__main__":
    d = np.load("/root/problem/inputs.npz")
    out = kernel(**{k: d[k] for k in d.files})
    ref = np.load("/root/problem/ref_out.npy")
    rel = np.abs(out - ref).max() / np.abs(ref).max()
    print("Relative error:", rel)

